# revision 8
# baseline (speedup 1.0000x reference)
"""Distributed attention kernel for 8 TRN2 NeuronCores.

Problem: x[2,2048,1024] -> qkv proj -> 16-head attention (softmax then /scale
quirk) -> out proj + bias.

Sharding: core c owns heads {2c, 2c+1} for BOTH batches. Restructured from the
349us baseline around three findings from its trace:

1. The attention inner loop is ScalarE(exp)-co-limited (exp [128,1024] =
   1147ns vs ~872ns of PE per k-chunk), so both heads are processed in ONE
   qh-block pass: qk logits for the two heads are computed with 64-row
   row-tiled matmuls at tile_position (0,0)/(64,0) that run CONCURRENTLY in
   the PE array (head dims live on partitions 0-63/64-127), writing one
   [128, 2x512] PSUM tile whose single exp covers both heads. score@v keeps
   the full-array padded stationary with the appended ones-column (softmax
   denominators for free) - col-tiling it would lose the ones column.
2. DMA was descriptor-bound (each 128-partition transfer costs ~144
   descriptors regardless of size): x loads per (batch,chunk) as one
   [128,2048] tile in 2 DMAs, w_qkv/w_out are host-packed so each is a
   SINGLE transfer. Baseline's 21us startup -> first matmul at ~10us.
3. The AllToAll now carries only useful data: per (batch, qpos-half) a
   [8 slots, 128 rows(=both local heads), 128 tokens] exchange fired
   mid-attention (after qh1) and at the end (qh3). Every core then projects
   2x128 tokens per batch against the full w_out - the baseline's
   discarded-half projections (27us of garbage PE work per core) are gone,
   A2A bytes halve, and w_out needs no host permutation (sender s's rows are
   heads 2s,2s+1 = w_out rows 128s..128s+128).

Projection work for batch b and the tail q-chains of batch b+1's QKV are
drained as fine-grained PE filler inside the ACT-limited attention loops.
All matmuls run as float32r; partial-row matmuls are only ever issued as
concurrent row-tiled pairs covering the full array (HAM clock-gate safety).
Do NOT mix bf16 and f32r matmuls - that produced nondeterministic weight
corruption on hardware in a previous session.
"""

import numpy as np

S = 2048          # sequence length
D = 1024          # model dim
NH = 16           # total heads
DH = 64           # head dim
HPC = 2           # heads per core
NCORES = 8
KC = 8            # k-chunks of D (128 each)
NK = S // 128     # kpos chunks per batch (16)
NQH = 4           # qpos blocks of 512 per batch
SCALE_INV = 8.0   # 1 / (DH ** -0.5)

USE_BF16 = False

_CACHE = {}


def _ensure_paths():
    import sys
    for p in ("/opt/trn_rl_repo", "/root/.axon_site"):
        if p not in sys.path:
            sys.path.insert(0, p)


def _build_nc():
    _ensure_paths()
    from contextlib import ExitStack
    import concourse.bass as bass
    import concourse.mybir as mybir
    import concourse.tile as tile
    from concourse import bacc
    from concourse.masks import make_identity

    f32 = mybir.dt.float32
    DT = mybir.dt.bfloat16 if USE_BF16 else mybir.dt.float32r
    DTT = mybir.dt.bfloat16 if USE_BF16 else f32  # transpose-path dtype
    EXP = mybir.ActivationFunctionType.Exp

    nc = bacc.Bacc(None)
    xT_ext = nc.declare_dram_parameter("xT", [2, KC, 128, S], DT, isOutput=False)
    wq_ext = nc.declare_dram_parameter("w_qkv", [128, KC * 3 * 128], DT, isOutput=False)
    wo_ext = nc.declare_dram_parameter("w_out", [128, KC * D], DT, isOutput=False)
    bout_ext = nc.declare_dram_parameter("b_out", [D], f32, isOutput=False)
    out_ext = nc.declare_dram_parameter("out", [2, 2, 128, D], f32, isOutput=True)

    with tile.TileContext(nc) as tc, ExitStack() as ctx:
        ctx.enter_context(
            nc.allow_low_precision(reason="f32r/bf16 storage throughout")
        )
        const = ctx.enter_context(tc.tile_pool(name="const", bufs=1))
        qk_pool = ctx.enter_context(tc.tile_pool(name="qk", bufs=4))
        vt_pool = ctx.enter_context(tc.tile_pool(name="vt", bufs=1))
        vo_pool = ctx.enter_context(tc.tile_pool(name="vo", bufs=32))
        st_pool = ctx.enter_context(tc.tile_pool(name="st", bufs=2))
        stage_pool = ctx.enter_context(tc.tile_pool(name="stg", bufs=2))
        ot_pool = ctx.enter_context(tc.tile_pool(name="ot", bufs=4))
        rcp_pool = ctx.enter_context(tc.tile_pool(name="rcp", bufs=2))
        bc_pool = ctx.enter_context(tc.tile_pool(name="bc", bufs=2))

        ps_lt = ctx.enter_context(tc.tile_pool(name="psLT", bufs=2, space="PSUM"))
        ps_ot = ctx.enter_context(tc.tile_pool(name="psOT", bufs=2, space="PSUM"))
        ps_a = ctx.enter_context(tc.tile_pool(name="psA", bufs=2, space="PSUM"))
        dram = ctx.enter_context(tc.tile_pool(name="dram", bufs=1, space="DRAM"))

        a2a_in = dram.tile([2, 2, NCORES, 128, 128], DT, tag="a2a_in", name="a2a_in")
        a2a_out = dram.tile([2, 2, NCORES, 128, 128], DT, tag="a2a_out", name="a2a_out")

        # ---- constants ----
        ident = const.tile([128, 128], DTT, tag="ident", name="ident")
        make_identity(nc, ident)
        ones2 = const.tile([128, HPC, 1], DTT, tag="ones2", name="ones2")
        nc.vector.memset(ones2, 1.0)
        zeros2 = const.tile([128, HPC, 128 - DH - 1], DTT, tag="zeros2", name="zeros2")
        nc.vector.memset(zeros2, 0.0)
        # pre-warm the exp table set so the ~2.7us ACT_TABLE_LOAD overlaps the
        # DMA-gated QKV phase instead of the first attention chunk
        warm = const.tile([1, 2], f32, tag="warm", name="warm")
        nc.vector.memset(warm, 0.0)
        nc.scalar.activation(warm, warm, EXP)

        def load_bias(pool):
            bias_sb = pool.tile([128, D], f32, tag="bias", name="bias_sb")
            bias_ap = bout_ext.ap()
            bias_bcast = bass.AP(
                tensor=bias_ap.tensor,
                offset=bias_ap.offset,
                ap=[[0, 128]] + [list(p) for p in bias_ap.ap],
            )
            nc.sync.dma_start(out=bias_sb, in_=bias_bcast)
            return bias_sb

        qT = {}
        kT = {}
        vo = {}
        stage = {}

        def drain(it, n=None):
            if it is None:
                return
            if n is None:
                for _ in it:
                    pass
            else:
                for _ in range(n):
                    if next(it, StopIteration) is StopIteration:
                        break

        def chain(*gens):
            for g in gens:
                if g is not None:
                    yield from g

        def load_x(b, xt_pool):
            # one [128,2048] tile per k-chunk; nkk0 quarter first so the
            # first accumulation chain's inputs lead the DMA queues
            xts = []
            for k in range(KC):
                t = xt_pool.tile([128, S], DT, tag="xt", name=f"xt{b}_{k}")
                xts.append(t)
            for k in range(KC):
                nc.sync.dma_start(out=xts[k][:, 0:512], in_=xT_ext[b, k][:, 0:512])
            for k in range(KC):
                nc.sync.dma_start(out=xts[k][:, 512:S], in_=xT_ext[b, k][:, 512:S])
            return xts

        def qkv_chains(b, wq_sb, xts, secs):
            # one generator step per matmul / copy; sections: 0=q, 1=k, 2=v
            for sec, nkks in secs:
                if sec == 0:
                    dst = qT[b]
                elif sec == 1:
                    dst = kT[b]
                else:
                    dst = vt_pool.tile([128, S], DTT, tag="vt", name=f"vT{b}")
                    vt_cur[b] = dst
                for nkk in nkks:
                    ps = ps_a.tile([128, 512], f32, tag="psA", name=f"qkv{b}_{sec}_{nkk}")
                    for k in range(KC):
                        nc.tensor.matmul(
                            ps,
                            lhsT=wq_sb[:, k * 384 + sec * 128:k * 384 + sec * 128 + 128],
                            rhs=xts[k][:, nkk * 512:(nkk + 1) * 512],
                            start=(k == 0),
                            stop=(k == KC - 1),
                        )
                        yield
                    nc.vector.tensor_copy(dst[:, nkk * 512:(nkk + 1) * 512], ps)
                    yield
                if sec == 2:
                    vT = vt_cur[b]
                    for sc in range(NK):
                        vps = ps_a.tile([128, 128], DTT, tag="psA", name=f"vps{b}_{sc}")
                        nc.tensor.transpose(vps, vT[:, sc * 128:(sc + 1) * 128], ident)
                        vt = vo_pool.tile([128, HPC, 128], DT, tag="vo", name=f"vo{b}_{sc}")
                        nc.vector.tensor_copy(
                            vt[:, :, 0:DH], vps.rearrange("p (h d) -> p h d", h=HPC)
                        )
                        nc.vector.tensor_copy(vt[:, :, DH:DH + 1], ones2)
                        nc.vector.tensor_copy(vt[:, :, DH + 1:], zeros2)
                        vo[b][sc] = vt
                        yield

        vt_cur = {}

        def qkv_start(b, wq_sb, xts):
            qT[b] = qk_pool.tile([128, S], DT, tag="qk", name=f"qT{b}")
            kT[b] = qk_pool.tile([128, S], DT, tag="qk", name=f"kT{b}")
            vo[b] = [None] * NK
            return qkv_chains(b, wq_sb, xts, [(1, range(4)), (2, range(4)), (0, range(4))])

        def attention(b, filler, rates, qh1_hook=None):
            stage[b] = stage_pool.tile([128, S], DT, tag="stg", name=f"stg{b}")
            for qh in range(NQH):
                q0 = qh * 512
                rate = rates[qh]
                outT = [
                    ps_ot.tile([128, 512], f32, tag="psOT", name=f"oT{b}_{qh}_{h}")
                    for h in range(HPC)
                ]

                def sv(k, st):
                    for h in range(HPC):
                        nc.tensor.matmul(
                            outT[h],
                            lhsT=vo[b][k][:, h, :],
                            rhs=st[:, h * 512:(h + 1) * 512],
                            start=(k == 0),
                            stop=(k == NK - 1),
                        )

                pending = None
                for k in range(NK):
                    lt = ps_lt.tile([128, 1024], f32, tag="psLT", name=f"lt{b}_{qh}_{k}")
                    for h in range(HPC):
                        nc.tensor.matmul(
                            lt[:, h * 512:(h + 1) * 512],
                            lhsT=kT[b][h * DH:(h + 1) * DH, k * 128:(k + 1) * 128],
                            rhs=qT[b][h * DH:(h + 1) * DH, q0:q0 + 512],
                            start=True,
                            stop=True,
                            tile_position=(h * DH, 0),
                        )
                    st = st_pool.tile([128, 1024], DT, tag="st", name=f"st{b}_{qh}_{k}")
                    nc.scalar.activation(st, lt, EXP)
                    if pending is not None:
                        sv(*pending)
                    pending = (k, st)
                    drain(filler, rate)
                sv(*pending)
                # normalize both heads into stage (evac on DVE, not ScalarE:
                # ScalarE is the exp-bottleneck engine)
                for h in range(HPC):
                    ot = ot_pool.tile([DH + 1, 512], f32, tag="ot", name=f"ot{b}_{qh}_{h}")
                    nc.vector.tensor_copy(ot, outT[h][0:DH + 1, :])
                    recip = rcp_pool.tile([1, 512], f32, tag="rcp", name=f"rcp{b}_{qh}_{h}")
                    nc.vector.reciprocal(recip, ot[DH:DH + 1, :])
                    bc = bc_pool.tile([DH, 512], f32, tag="bc", name=f"bc{b}_{qh}_{h}")
                    nc.gpsimd.partition_broadcast(bc, recip)
                    nc.vector.scalar_tensor_tensor(
                        out=stage[b][h * DH:(h + 1) * DH, q0:q0 + 512],
                        in0=ot[0:DH, :],
                        scalar=SCALE_INV,
                        in1=bc,
                        op0=mybir.AluOpType.mult,
                        op1=mybir.AluOpType.mult,
                    )
                if qh in (1, 3):
                    half = qh // 2
                    for s in range(NCORES):
                        nc.gpsimd.dma_start(
                            out=a2a_in[b, half, s],
                            in_=stage[b][:, half * 1024 + s * 128:half * 1024 + (s + 1) * 128],
                        )
                    nc.gpsimd.collective_compute(
                        "AllToAll",
                        mybir.AluOpType.bypass,
                        replica_groups=[list(range(NCORES))],
                        ins=[a2a_in[b, half].opt()],
                        outs=[a2a_out[b, half].opt()],
                    )
                    if qh == 1 and qh1_hook is not None:
                        filler = chain(filler, qh1_hook())

        def proj_gen(b, half, wo_sb, bias_sb, g_pool, y_pool):
            g_sb = []
            for s in range(NCORES):
                t = g_pool.tile([128, 128], DT, tag="g", name=f"g{b}_{half}_{s}")
                nc.sync.dma_start(out=t, in_=a2a_out[b, half, s])
                g_sb.append(t)

            def gen():
                y_sb = y_pool.tile([128, D], f32, tag="y", name=f"y{b}_{half}")
                for nk in range(2):
                    yps = ps_a.tile([128, 512], f32, tag="psA", name=f"yps{b}_{half}_{nk}")
                    for s in range(NCORES):
                        nc.tensor.matmul(
                            yps,
                            lhsT=g_sb[s],
                            rhs=wo_sb[:, s * D + nk * 512:s * D + (nk + 1) * 512],
                            start=(s == 0),
                            stop=(s == NCORES - 1),
                        )
                        yield
                    nc.vector.tensor_add(
                        y_sb[:, nk * 512:(nk + 1) * 512],
                        yps,
                        bias_sb[:, nk * 512:(nk + 1) * 512],
                    )
                    yield
                nc.sync.dma_start(out=out_ext[b, half], in_=y_sb)
                yield

            return gen()

        with tc.tile_pool(name="xt", bufs=10) as xt_pool, \
                tc.tile_pool(name="wq", bufs=1) as wq_pool:
            wq_sb = wq_pool.tile([128, KC * 3 * 128], DT, tag="wq", name="wq")
            nc.sync.dma_start(out=wq_sb, in_=wq_ext.ap())
            xts0 = load_x(0, xt_pool)
            drain(qkv_start(0, wq_sb, xts0))
            xts1 = load_x(1, xt_pool)
            g1 = qkv_start(1, wq_sb, xts1)
            # qh0 rate 0: b1's x tiles reuse b0's slots, so its first filler
            # matmuls are DMA-gated for ~the first qh block - don't let them
            # block the attention stream
            attention(0, g1, rates=(0, 5, 5, 5))
            drain(g1)
        # xt/wq freed: projection pools fit alongside the attention pools
        wo_pool = ctx.enter_context(tc.tile_pool(name="wo", bufs=1))
        g_pool = ctx.enter_context(tc.tile_pool(name="g", bufs=4 * NCORES))
        y_pool = ctx.enter_context(tc.tile_pool(name="y", bufs=2))
        bias_sb = load_bias(y_pool)
        wo_sb = wo_pool.tile([128, KC * D], DT, tag="wo", name="wo")
        nc.sync.dma_start(out=wo_sb, in_=wo_ext.ap())
        p0 = chain(proj_gen(0, 0, wo_sb, bias_sb, g_pool, y_pool),
                   proj_gen(0, 1, wo_sb, bias_sb, g_pool, y_pool))
        attention(
            1, p0, rates=(1, 1, 1, 1),
            qh1_hook=lambda: proj_gen(1, 0, wo_sb, bias_sb, g_pool, y_pool),
        )
        drain(p0)
        drain(proj_gen(1, 1, wo_sb, bias_sb, g_pool, y_pool))

    nc.finalize()
    return nc


def _prep_in_maps(x, w_qkv, w_out, b_out):
    if USE_BF16:
        import ml_dtypes
        dt = ml_dtypes.bfloat16
    else:
        dt = np.float32
    x = np.asarray(x, dtype=np.float32)
    w_qkv = np.asarray(w_qkv, dtype=np.float32)
    w_out = np.asarray(w_out, dtype=np.float32)
    b_out = np.ascontiguousarray(b_out, dtype=np.float32)

    xT = np.ascontiguousarray(
        np.stack([x[0].T, x[1].T]).reshape(2, KC, 128, S).astype(dt)
    )
    # w_out rows grouped per sender s (heads 2s, 2s+1) = natural row order,
    # packed so the whole thing is ONE [128, 8192] transfer
    wo = np.ascontiguousarray(
        w_out.reshape(KC, 128, D).transpose(1, 0, 2).reshape(128, KC * D).astype(dt)
    )
    in_maps = []
    for c in range(NCORES):
        c0 = c * HPC * DH
        shard = np.concatenate(
            [
                w_qkv[:, c0:c0 + 128],
                w_qkv[:, D + c0:D + c0 + 128],
                w_qkv[:, 2 * D + c0:2 * D + c0 + 128],
            ],
            axis=1,
        )  # [1024, 384]
        wq = np.ascontiguousarray(
            shard.reshape(KC, 128, 3 * 128).transpose(1, 0, 2).reshape(128, -1).astype(dt)
        )
        in_maps.append({"xT": xT, "w_qkv": wq, "w_out": wo, "b_out": b_out})
    return in_maps


def _run(x, w_qkv, w_out, b_out, trace=False):
    _ensure_paths()
    from concourse.bass_utils import run_bass_kernel_spmd

    if "nc" not in _CACHE:
        _CACHE["nc"] = _build_nc()
    nc = _CACHE["nc"]
    in_maps = _prep_in_maps(x, w_qkv, w_out, b_out)
    res = run_bass_kernel_spmd(nc, in_maps, list(range(NCORES)), trace=trace)
    out = np.empty((2, S, D), dtype=np.float32)
    for c in range(NCORES):
        o = np.asarray(res.results[c]["out"], dtype=np.float32)
        for b in range(2):
            for half in range(2):
                t0 = half * 1024 + c * 128
                out[b, t0:t0 + 128, :] = o[b, half]
    return out, res


def kernel(x, w_qkv, w_out, b_out):
    out, _ = _run(x, w_qkv, w_out, b_out, trace=False)
    return out


# revision 9
# speedup vs baseline: 1.3533x; 1.3533x over previous
"""Distributed attention kernel for 8 TRN2 NeuronCores.

Problem: x[2,2048,1024] -> qkv proj -> 16-head attention (softmax then /scale
quirk) -> out proj + bias.

Sharding: core c owns heads {2c, 2c+1} for BOTH batches. Restructured from the
349us baseline around three findings from its trace:

1. The attention inner loop is ScalarE(exp)-co-limited (exp [128,1024] =
   1147ns vs ~872ns of PE per k-chunk), so both heads are processed in ONE
   qh-block pass: qk logits for the two heads are computed with 64-row
   row-tiled matmuls at tile_position (0,0)/(64,0) that run CONCURRENTLY in
   the PE array (head dims live on partitions 0-63/64-127), writing one
   [128, 2x512] PSUM tile whose single exp covers both heads. score@v keeps
   the full-array padded stationary with the appended ones-column (softmax
   denominators for free) - col-tiling it would lose the ones column.
2. DMA was descriptor-bound (each 128-partition transfer costs ~144
   descriptors regardless of size): x loads per (batch,chunk) as one
   [128,2048] tile in 2 DMAs, w_qkv/w_out are host-packed so each is a
   SINGLE transfer. Baseline's 21us startup -> first matmul at ~10us.
3. The AllToAll now carries only useful data: per (batch, qpos-half) a
   [8 slots, 128 rows(=both local heads), 128 tokens] exchange fired
   mid-attention (after qh1) and at the end (qh3). Every core then projects
   2x128 tokens per batch against the full w_out - the baseline's
   discarded-half projections (27us of garbage PE work per core) are gone,
   A2A bytes halve, and w_out needs no host permutation (sender s's rows are
   heads 2s,2s+1 = w_out rows 128s..128s+128).

Projection work for batch b and the tail q-chains of batch b+1's QKV are
drained as fine-grained PE filler inside the ACT-limited attention loops.
All matmuls run as float32r; partial-row matmuls are only ever issued as
concurrent row-tiled pairs covering the full array (HAM clock-gate safety).
Do NOT mix bf16 and f32r matmuls - that produced nondeterministic weight
corruption on hardware in a previous session.
"""

import numpy as np

S = 2048          # sequence length
D = 1024          # model dim
NH = 16           # total heads
DH = 64           # head dim
HPC = 2           # heads per core
NCORES = 8
KC = 8            # k-chunks of D (128 each)
NK = S // 128     # kpos chunks per batch (16)
NQH = 4           # qpos blocks of 512 per batch
SCALE_INV = 8.0   # 1 / (DH ** -0.5)

USE_BF16 = True
ROW_TILE_QK = True

_CACHE = {}


def _ensure_paths():
    import sys
    for p in ("/opt/trn_rl_repo", "/root/.axon_site"):
        if p not in sys.path:
            sys.path.insert(0, p)


def _build_nc():
    _ensure_paths()
    from contextlib import ExitStack
    import concourse.bass as bass
    import concourse.mybir as mybir
    import concourse.tile as tile
    from concourse import bacc
    from concourse.masks import make_identity

    f32 = mybir.dt.float32
    DT = mybir.dt.bfloat16 if USE_BF16 else mybir.dt.float32r
    DTT = mybir.dt.bfloat16 if USE_BF16 else f32  # transpose-path dtype
    EXP = mybir.ActivationFunctionType.Exp

    nc = bacc.Bacc(None)
    xT_ext = nc.declare_dram_parameter("xT", [2, KC, 128, S], DT, isOutput=False)
    wq_ext = nc.declare_dram_parameter("w_qkv", [128, KC * 3 * 128], DT, isOutput=False)
    wo_ext = nc.declare_dram_parameter("w_out", [128, KC * D], DT, isOutput=False)
    bout_ext = nc.declare_dram_parameter("b_out", [D], f32, isOutput=False)
    out_ext = nc.declare_dram_parameter("out", [2, 2, 128, D], f32, isOutput=True)

    with tile.TileContext(nc) as tc, ExitStack() as ctx:
        ctx.enter_context(
            nc.allow_low_precision(reason="f32r/bf16 storage throughout")
        )
        const = ctx.enter_context(tc.tile_pool(name="const", bufs=1))
        qk_pool = ctx.enter_context(tc.tile_pool(name="qk", bufs=4))
        vt_pool = ctx.enter_context(tc.tile_pool(name="vt", bufs=1))
        vo_pool = ctx.enter_context(tc.tile_pool(name="vo", bufs=32))
        st_pool = ctx.enter_context(tc.tile_pool(name="st", bufs=2))
        stage_pool = ctx.enter_context(tc.tile_pool(name="stg", bufs=2))
        ot_pool = ctx.enter_context(tc.tile_pool(name="ot", bufs=4))
        rcp_pool = ctx.enter_context(tc.tile_pool(name="rcp", bufs=2))
        bc_pool = ctx.enter_context(tc.tile_pool(name="bc", bufs=2))

        ps_lt = ctx.enter_context(tc.tile_pool(name="psLT", bufs=2, space="PSUM"))
        ps_ot = ctx.enter_context(tc.tile_pool(name="psOT", bufs=2, space="PSUM"))
        ps_a = ctx.enter_context(tc.tile_pool(name="psA", bufs=2, space="PSUM"))
        dram = ctx.enter_context(tc.tile_pool(name="dram", bufs=1, space="DRAM"))

        a2a_in = dram.tile([2, 2, NCORES, 128, 128], DT, tag="a2a_in", name="a2a_in")
        a2a_out = dram.tile([2, 2, NCORES, 128, 128], DT, tag="a2a_out", name="a2a_out")
        cc_warm_in = dram.tile([NCORES, 128], DT, tag="ccw_i", name="ccw_i")
        cc_warm_out = dram.tile([NCORES, 128], DT, tag="ccw_o", name="ccw_o")

        # ---- constants ----
        ident = const.tile([128, 128], DTT, tag="ident", name="ident")
        make_identity(nc, ident)
        ones2 = const.tile([128, HPC, 1], DTT, tag="ones2", name="ones2")
        nc.vector.memset(ones2, 1.0)
        zeros2 = const.tile([128, HPC, 128 - DH - 1], DTT, tag="zeros2", name="zeros2")
        nc.vector.memset(zeros2, 0.0)
        zpad = const.tile([DH, 512], DT, tag="zpad", name="zpad")
        zscr = const.tile([DH, 512], f32, tag="zscr", name="zscr")
        nc.vector.memset(zscr, 0.0)
        nc.vector.tensor_copy(zpad, zscr)
        # pre-warm the exp table set so the ~2.7us ACT_TABLE_LOAD overlaps the
        # DMA-gated QKV phase instead of the first attention chunk
        warm = const.tile([1, 2], f32, tag="warm", name="warm")
        nc.vector.memset(warm, 0.0)
        nc.scalar.activation(warm, warm, EXP)

        def load_bias(pool):
            bias_sb = pool.tile([128, D], f32, tag="bias", name="bias_sb")
            bias_ap = bout_ext.ap()
            bias_bcast = bass.AP(
                tensor=bias_ap.tensor,
                offset=bias_ap.offset,
                ap=[[0, 128]] + [list(p) for p in bias_ap.ap],
            )
            nc.sync.dma_start(out=bias_sb, in_=bias_bcast)
            return bias_sb

        qT = {}
        kT = {}
        vo = {}
        stage = {}

        def drain(it, n=None):
            if it is None:
                return
            if n is None:
                for _ in it:
                    pass
            else:
                for _ in range(n):
                    if next(it, StopIteration) is StopIteration:
                        break

        def chain(*gens):
            for g in gens:
                if g is not None:
                    yield from g

        def load_x(b, xt_pool):
            # one [128,2048] tile per k-chunk, filled in nkk-major waves so
            # the n-th accumulation chain only waits for the n-th wave
            xts = []
            for k in range(KC):
                t = xt_pool.tile([128, S], DT, tag="xt", name=f"xt{b}_{k}")
                xts.append(t)
            for nkk in range(4):
                for k in range(KC):
                    nc.sync.dma_start(
                        out=xts[k][:, nkk * 512:(nkk + 1) * 512],
                        in_=xT_ext[b, k][:, nkk * 512:(nkk + 1) * 512],
                    )
            return xts

        def qkv_chains(b, wq_sb, xts, secs):
            # one generator step per matmul / copy; sections: 0=q, 1=k, 2=v
            for sec, nkks in secs:
                if sec == 0:
                    dst = qT[b]
                elif sec == 1:
                    dst = kT[b]
                else:
                    dst = vt_pool.tile([128, S], DTT, tag="vt", name=f"vT{b}")
                    vt_cur[b] = dst
                for nkk in nkks:
                    ps = ps_a.tile([128, 512], f32, tag="psA", name=f"qkv{b}_{sec}_{nkk}")
                    for k in range(KC):
                        nc.tensor.matmul(
                            ps,
                            lhsT=wq_sb[:, k * 384 + sec * 128:k * 384 + sec * 128 + 128],
                            rhs=xts[k][:, nkk * 512:(nkk + 1) * 512],
                            start=(k == 0),
                            stop=(k == KC - 1),
                        )
                        yield
                    if sec == 0 and not ROW_TILE_QK:
                        c0 = nkk * 512
                        for h in range(HPC):
                            nc.vector.tensor_copy(
                                qT[b][h][h * DH:(h + 1) * DH, c0:c0 + 512],
                                ps[h * DH:(h + 1) * DH, :],
                            )
                    else:
                        nc.vector.tensor_copy(dst[:, nkk * 512:(nkk + 1) * 512], ps)
                    yield
                if sec == 2:
                    vT = vt_cur[b]
                    for sc in range(NK):
                        vps = ps_a.tile([128, 128], DTT, tag="psA", name=f"vps{b}_{sc}")
                        nc.tensor.transpose(vps, vT[:, sc * 128:(sc + 1) * 128], ident)
                        vt = vo_pool.tile([128, HPC, 128], DT, tag="vo", name=f"vo{b}_{sc}")
                        nc.vector.tensor_copy(
                            vt[:, :, 0:DH], vps.rearrange("p (h d) -> p h d", h=HPC)
                        )
                        nc.vector.tensor_copy(vt[:, :, DH:DH + 1], ones2)
                        nc.vector.tensor_copy(vt[:, :, DH + 1:], zeros2)
                        vo[b][sc] = vt
                        yield

        vt_cur = {}

        def qkv_start(b, wq_sb, xts):
            if ROW_TILE_QK:
                qT[b] = qk_pool.tile([128, S], DT, tag="qk", name=f"qT{b}")
            else:
                qT[b] = [
                    qk_pool.tile([128, S], DT, tag="qk", name=f"qT{b}_{h}")
                    for h in range(HPC)
                ]
                for h in range(HPC):
                    r0 = DH * (1 - h)
                    for c in range(4):
                        nc.vector.tensor_copy(
                            qT[b][h][r0:r0 + DH, c * 512:(c + 1) * 512], zpad
                        )
            kT[b] = qk_pool.tile([128, S], DT, tag="qk", name=f"kT{b}")
            vo[b] = [None] * NK
            return qkv_chains(b, wq_sb, xts, [(1, range(4)), (2, range(4)), (0, range(4))])

        def attention(b, fillers, rates):
            # fillers: {qh: generator} appended to the live filler at that block
            live = []
            stage[b] = stage_pool.tile([128, S], DT, tag="stg", name=f"stg{b}")

            def filler_step(n):
                for _ in range(n):
                    while live:
                        if next(live[0], StopIteration) is StopIteration:
                            live.pop(0)
                        else:
                            break
                    if not live:
                        return

            for qh in range(NQH):
                q0 = qh * 512
                rate = rates[qh]
                if fillers.get(qh) is not None:
                    live.append(fillers[qh])
                outT = [
                    ps_ot.tile([128, 512], f32, tag="psOT", name=f"oT{b}_{qh}_{h}")
                    for h in range(HPC)
                ]

                def sv(k, st):
                    for h in range(HPC):
                        nc.tensor.matmul(
                            outT[h],
                            lhsT=vo[b][k][:, h, :],
                            rhs=st[:, h * 512:(h + 1) * 512],
                            start=(k == 0),
                            stop=(k == NK - 1),
                        )

                pending = None
                for k in range(NK):
                    lt = ps_lt.tile([128, 1024], f32, tag="psLT", name=f"lt{b}_{qh}_{k}")
                    for h in range(HPC):
                        if ROW_TILE_QK:
                            nc.tensor.matmul(
                                lt[:, h * 512:(h + 1) * 512],
                                lhsT=kT[b][h * DH:(h + 1) * DH, k * 128:(k + 1) * 128],
                                rhs=qT[b][h * DH:(h + 1) * DH, q0:q0 + 512],
                                start=True,
                                stop=True,
                                tile_position=(h * DH, 0),
                            )
                        else:
                            nc.tensor.matmul(
                                lt[:, h * 512:(h + 1) * 512],
                                lhsT=kT[b][:, k * 128:(k + 1) * 128],
                                rhs=qT[b][h][:, q0:q0 + 512],
                                start=True,
                                stop=True,
                            )
                    st = st_pool.tile([128, 1024], DT, tag="st", name=f"st{b}_{qh}_{k}")
                    nc.scalar.activation(st, lt, EXP)
                    if pending is not None:
                        sv(*pending)
                    pending = (k, st)
                    filler_step(rate)
                sv(*pending)
                # normalize both heads into stage (evac on DVE, not ScalarE:
                # ScalarE is the exp-bottleneck engine)
                for h in range(HPC):
                    ot = ot_pool.tile([DH + 1, 512], f32, tag="ot", name=f"ot{b}_{qh}_{h}")
                    nc.vector.tensor_copy(ot, outT[h][0:DH + 1, :])
                    recip = rcp_pool.tile([1, 512], f32, tag="rcp", name=f"rcp{b}_{qh}_{h}")
                    nc.vector.reciprocal(recip, ot[DH:DH + 1, :])
                    bc = bc_pool.tile([DH, 512], f32, tag="bc", name=f"bc{b}_{qh}_{h}")
                    nc.gpsimd.partition_broadcast(bc, recip)
                    nc.vector.scalar_tensor_tensor(
                        out=stage[b][h * DH:(h + 1) * DH, q0:q0 + 512],
                        in0=ot[0:DH, :],
                        scalar=SCALE_INV,
                        in1=bc,
                        op0=mybir.AluOpType.mult,
                        op1=mybir.AluOpType.mult,
                    )
                if qh in (1, 3):
                    half = qh // 2
                    for s in range(NCORES):
                        nc.gpsimd.dma_start(
                            out=a2a_in[b, half, s],
                            in_=stage[b][:, half * 1024 + s * 128:half * 1024 + (s + 1) * 128],
                        )
                    nc.gpsimd.collective_compute(
                        "AllToAll",
                        mybir.AluOpType.bypass,
                        replica_groups=[list(range(NCORES))],
                        ins=[a2a_in[b, half].opt()],
                        outs=[a2a_out[b, half].opt()],
                    )

        def proj_gen(b, half, wo_sb, bias_sb, g_pool, y_pool):
            g_sb = []
            for s in range(NCORES):
                t = g_pool.tile([128, 128], DT, tag="g", name=f"g{b}_{half}_{s}")
                nc.sync.dma_start(out=t, in_=a2a_out[b, half, s])
                g_sb.append(t)

            def gen():
                y_sb = y_pool.tile([128, D], f32, tag="y", name=f"y{b}_{half}")
                for nk in range(2):
                    yps = ps_a.tile([128, 512], f32, tag="psA", name=f"yps{b}_{half}_{nk}")
                    for s in range(NCORES):
                        nc.tensor.matmul(
                            yps,
                            lhsT=g_sb[s],
                            rhs=wo_sb[:, s * D + nk * 512:s * D + (nk + 1) * 512],
                            start=(s == 0),
                            stop=(s == NCORES - 1),
                        )
                        yield
                    nc.vector.tensor_add(
                        y_sb[:, nk * 512:(nk + 1) * 512],
                        yps,
                        bias_sb[:, nk * 512:(nk + 1) * 512],
                    )
                    yield
                nc.sync.dma_start(out=out_ext[b, half], in_=y_sb)
                yield

            return gen()

        with tc.tile_pool(name="xt", bufs=16) as xt_pool, \
                tc.tile_pool(name="wq", bufs=1) as wq_pool:
            wq_sb = wq_pool.tile([128, KC * 3 * 128], DT, tag="wq", name="wq")
            nc.sync.dma_start(out=wq_sb, in_=wq_ext.ap())
            # tiny throwaway AllToAll: the first collective pays ~30us of
            # cold-start; absorb it under the DMA-gated QKV phase
            nc.gpsimd.dma_start(out=cc_warm_in[0:1], in_=wq_sb[0:1, 0:128])
            nc.gpsimd.collective_compute(
                "AllToAll",
                mybir.AluOpType.bypass,
                replica_groups=[list(range(NCORES))],
                ins=[cc_warm_in.opt()],
                outs=[cc_warm_out.opt()],
            )
            xts0 = load_x(0, xt_pool)
            drain(qkv_start(0, wq_sb, xts0))
            xts1 = load_x(1, xt_pool)
            g1 = qkv_start(1, wq_sb, xts1)
            attention(0, {0: g1}, rates=(3, 3, 3, 3))
            drain(g1)
        # xt/wq freed: projection pools fit alongside the attention pools
        wo_pool = ctx.enter_context(tc.tile_pool(name="wo", bufs=1))
        g_pool = ctx.enter_context(tc.tile_pool(name="g", bufs=4 * NCORES))
        y_pool = ctx.enter_context(tc.tile_pool(name="y", bufs=2))
        bias_sb = load_bias(y_pool)
        wo_sb = wo_pool.tile([128, KC * D], DT, tag="wo", name="wo")
        nc.sync.dma_start(out=wo_sb, in_=wo_ext.ap())
        # proj(b0) half0's A2A completed mid-attn(0); half1's completes
        # ~20us into attn(1) - only queue its matmuls from qh2 on so the PE
        # stream never blocks on an in-flight collective
        attention(1, {
            0: proj_gen(0, 0, wo_sb, bias_sb, g_pool, y_pool),
            2: proj_gen(0, 1, wo_sb, bias_sb, g_pool, y_pool),
        }, rates=(1, 1, 1, 1))
        drain(proj_gen(1, 0, wo_sb, bias_sb, g_pool, y_pool))
        drain(proj_gen(1, 1, wo_sb, bias_sb, g_pool, y_pool))

    nc.finalize()
    return nc


def _prep_in_maps(x, w_qkv, w_out, b_out):
    if USE_BF16:
        import ml_dtypes
        dt = ml_dtypes.bfloat16
    else:
        dt = np.float32
    x = np.asarray(x, dtype=np.float32)
    w_qkv = np.asarray(w_qkv, dtype=np.float32)
    w_out = np.asarray(w_out, dtype=np.float32)
    b_out = np.ascontiguousarray(b_out, dtype=np.float32)

    xT = np.ascontiguousarray(
        np.stack([x[0].T, x[1].T]).reshape(2, KC, 128, S).astype(dt)
    )
    # w_out rows grouped per sender s (heads 2s, 2s+1) = natural row order,
    # packed so the whole thing is ONE [128, 8192] transfer
    wo = np.ascontiguousarray(
        w_out.reshape(KC, 128, D).transpose(1, 0, 2).reshape(128, KC * D).astype(dt)
    )
    in_maps = []
    for c in range(NCORES):
        c0 = c * HPC * DH
        shard = np.concatenate(
            [
                w_qkv[:, c0:c0 + 128],
                w_qkv[:, D + c0:D + c0 + 128],
                w_qkv[:, 2 * D + c0:2 * D + c0 + 128],
            ],
            axis=1,
        )  # [1024, 384]
        wq = np.ascontiguousarray(
            shard.reshape(KC, 128, 3 * 128).transpose(1, 0, 2).reshape(128, -1).astype(dt)
        )
        in_maps.append({"xT": xT, "w_qkv": wq, "w_out": wo, "b_out": b_out})
    return in_maps


def _run(x, w_qkv, w_out, b_out, trace=False):
    _ensure_paths()
    from concourse.bass_utils import run_bass_kernel_spmd

    if "nc" not in _CACHE:
        _CACHE["nc"] = _build_nc()
    nc = _CACHE["nc"]
    in_maps = _prep_in_maps(x, w_qkv, w_out, b_out)
    res = run_bass_kernel_spmd(nc, in_maps, list(range(NCORES)), trace=trace)
    out = np.empty((2, S, D), dtype=np.float32)
    for c in range(NCORES):
        o = np.asarray(res.results[c]["out"], dtype=np.float32)
        for b in range(2):
            for half in range(2):
                t0 = half * 1024 + c * 128
                out[b, t0:t0 + 128, :] = o[b, half]
    return out, res


def kernel(x, w_qkv, w_out, b_out):
    out, _ = _run(x, w_qkv, w_out, b_out, trace=False)
    return out


# revision 11
# speedup vs baseline: 1.3972x; 1.0324x over previous
"""Distributed attention kernel for 8 TRN2 NeuronCores.

Problem: x[2,2048,1024] -> qkv proj -> 16-head attention (softmax then /scale
quirk) -> out proj + bias.

Sharding: core c owns heads {2c, 2c+1} for BOTH batches. Restructured from the
349us baseline around three findings from its trace:

1. The attention inner loop is ScalarE(exp)-co-limited (exp [128,1024] =
   1147ns vs ~872ns of PE per k-chunk), so both heads are processed in ONE
   qh-block pass: qk logits for the two heads are computed with 64-row
   row-tiled matmuls at tile_position (0,0)/(64,0) that run CONCURRENTLY in
   the PE array (head dims live on partitions 0-63/64-127), writing one
   [128, 2x512] PSUM tile whose single exp covers both heads. score@v keeps
   the full-array padded stationary with the appended ones-column (softmax
   denominators for free) - col-tiling it would lose the ones column.
2. DMA was descriptor-bound (each 128-partition transfer costs ~144
   descriptors regardless of size): x loads per (batch,chunk) as one
   [128,2048] tile in 2 DMAs, w_qkv/w_out are host-packed so each is a
   SINGLE transfer. Baseline's 21us startup -> first matmul at ~10us.
3. The AllToAll now carries only useful data: per (batch, qpos-half) a
   [8 slots, 128 rows(=both local heads), 128 tokens] exchange fired
   mid-attention (after qh1) and at the end (qh3). Every core then projects
   2x128 tokens per batch against the full w_out - the baseline's
   discarded-half projections (27us of garbage PE work per core) are gone,
   A2A bytes halve, and w_out needs no host permutation (sender s's rows are
   heads 2s,2s+1 = w_out rows 128s..128s+128).

Projection work for batch b and the tail q-chains of batch b+1's QKV are
drained as fine-grained PE filler inside the ACT-limited attention loops.
All matmuls run as float32r; partial-row matmuls are only ever issued as
concurrent row-tiled pairs covering the full array (HAM clock-gate safety).
Do NOT mix bf16 and f32r matmuls - that produced nondeterministic weight
corruption on hardware in a previous session.
"""

import numpy as np

S = 2048          # sequence length
D = 1024          # model dim
NH = 16           # total heads
DH = 64           # head dim
HPC = 2           # heads per core
NCORES = 8
KC = 8            # k-chunks of D (128 each)
NK = S // 128     # kpos chunks per batch (16)
NQH = 4           # qpos blocks of 512 per batch
SCALE_INV = 8.0   # 1 / (DH ** -0.5)

USE_BF16 = True
ROW_TILE_QK = True

_CACHE = {}


def _ensure_paths():
    import sys
    for p in ("/opt/trn_rl_repo", "/root/.axon_site"):
        if p not in sys.path:
            sys.path.insert(0, p)


def _build_nc():
    _ensure_paths()
    from contextlib import ExitStack
    import concourse.bass as bass
    import concourse.mybir as mybir
    import concourse.tile as tile
    from concourse import bacc
    from concourse.masks import make_identity

    f32 = mybir.dt.float32
    DT = mybir.dt.bfloat16 if USE_BF16 else mybir.dt.float32r
    DTT = mybir.dt.bfloat16 if USE_BF16 else f32  # transpose-path dtype
    EXP = mybir.ActivationFunctionType.Exp

    nc = bacc.Bacc(None)
    xT_ext = nc.declare_dram_parameter("xT", [2, KC, 128, S], DT, isOutput=False)
    wq_ext = nc.declare_dram_parameter("w_qkv", [128, KC * 3 * 128], DT, isOutput=False)
    wo_ext = nc.declare_dram_parameter("w_out", [128, KC * D], DT, isOutput=False)
    bout_ext = nc.declare_dram_parameter("b_out", [D], f32, isOutput=False)
    out_ext = nc.declare_dram_parameter("out", [2, 2, 128, D], f32, isOutput=True)

    with tile.TileContext(nc) as tc, ExitStack() as ctx:
        ctx.enter_context(
            nc.allow_low_precision(reason="f32r/bf16 storage throughout")
        )
        const = ctx.enter_context(tc.tile_pool(name="const", bufs=1))
        qk_pool = ctx.enter_context(tc.tile_pool(name="qk", bufs=4))
        vt_pool = ctx.enter_context(tc.tile_pool(name="vt", bufs=1))
        vo_pool = ctx.enter_context(tc.tile_pool(name="vo", bufs=32))
        st_pool = ctx.enter_context(tc.tile_pool(name="st", bufs=2))
        stage_pool = ctx.enter_context(tc.tile_pool(name="stg", bufs=2))
        ot_pool = ctx.enter_context(tc.tile_pool(name="ot", bufs=4))
        rcp_pool = ctx.enter_context(tc.tile_pool(name="rcp", bufs=2))
        bc_pool = ctx.enter_context(tc.tile_pool(name="bc", bufs=2))

        ps_lt = ctx.enter_context(tc.tile_pool(name="psLT", bufs=2, space="PSUM"))
        ps_ot = ctx.enter_context(tc.tile_pool(name="psOT", bufs=2, space="PSUM"))
        ps_a = ctx.enter_context(tc.tile_pool(name="psA", bufs=2, space="PSUM"))
        dram = ctx.enter_context(tc.tile_pool(name="dram", bufs=1, space="DRAM"))

        a2a_in = {}
        a2a_out = {}
        for bb in range(2):
            for hh in range(2):
                a2a_in[bb, hh] = dram.tile(
                    [NCORES, 128, 128], DT, tag=f"a2ai{bb}{hh}", name=f"a2a_in{bb}{hh}"
                )
                a2a_out[bb, hh] = dram.tile(
                    [NCORES, 128, 128], DT, tag=f"a2ao{bb}{hh}", name=f"a2a_out{bb}{hh}"
                )
        cc_warm_in = dram.tile([NCORES, 128], DT, tag="ccw_i", name="ccw_i")
        cc_warm_out = dram.tile([NCORES, 128], DT, tag="ccw_o", name="ccw_o")

        # ---- constants ----
        ident = const.tile([128, 128], DTT, tag="ident", name="ident")
        make_identity(nc, ident)
        ones2 = const.tile([128, HPC, 1], DTT, tag="ones2", name="ones2")
        nc.vector.memset(ones2, 1.0)
        zeros2 = const.tile([128, HPC, 128 - DH - 1], DTT, tag="zeros2", name="zeros2")
        nc.vector.memset(zeros2, 0.0)
        zpad = const.tile([DH, 512], DT, tag="zpad", name="zpad")
        zscr = const.tile([DH, 512], f32, tag="zscr", name="zscr")
        nc.vector.memset(zscr, 0.0)
        nc.vector.tensor_copy(zpad, zscr)
        # pre-warm the exp table set so the ~2.7us ACT_TABLE_LOAD overlaps the
        # DMA-gated QKV phase instead of the first attention chunk
        warm = const.tile([1, 2], f32, tag="warm", name="warm")
        nc.vector.memset(warm, 0.0)
        nc.scalar.activation(warm, warm, EXP)

        def load_bias(pool):
            bias_sb = pool.tile([128, D], f32, tag="bias", name="bias_sb")
            bias_ap = bout_ext.ap()
            bias_bcast = bass.AP(
                tensor=bias_ap.tensor,
                offset=bias_ap.offset,
                ap=[[0, 128]] + [list(p) for p in bias_ap.ap],
            )
            nc.sync.dma_start(out=bias_sb, in_=bias_bcast)
            return bias_sb

        qT = {}
        kT = {}
        vo = {}
        stage = {}

        def drain(it, n=None):
            if it is None:
                return
            if n is None:
                for _ in it:
                    pass
            else:
                for _ in range(n):
                    if next(it, StopIteration) is StopIteration:
                        break

        def chain(*gens):
            for g in gens:
                if g is not None:
                    yield from g

        def load_x(b, xt_pool):
            # one [128,2048] tile per k-chunk, filled in nkk-major waves so
            # the n-th accumulation chain only waits for the n-th wave
            xts = []
            for k in range(KC):
                t = xt_pool.tile([128, S], DT, tag="xt", name=f"xt{b}_{k}")
                xts.append(t)
            for k in range(KC):
                nc.sync.dma_start(out=xts[k], in_=xT_ext[b, k])
            return xts

        def qkv_chains(b, wq_sb, xts, secs):
            # one generator step per matmul / copy; sections: 0=q, 1=k, 2=v
            for sec, nkks in secs:
                if sec == 0:
                    dst = qT[b]
                elif sec == 1:
                    dst = kT[b]
                else:
                    dst = vt_pool.tile([128, S], DTT, tag="vt", name=f"vT{b}")
                    vt_cur[b] = dst
                for nkk in nkks:
                    ps = ps_a.tile([128, 512], f32, tag="psA", name=f"qkv{b}_{sec}_{nkk}")
                    for k in range(KC):
                        nc.tensor.matmul(
                            ps,
                            lhsT=wq_sb[:, k * 384 + sec * 128:k * 384 + sec * 128 + 128],
                            rhs=xts[k][:, nkk * 512:(nkk + 1) * 512],
                            start=(k == 0),
                            stop=(k == KC - 1),
                        )
                        yield
                    if sec == 0 and not ROW_TILE_QK:
                        c0 = nkk * 512
                        for h in range(HPC):
                            nc.vector.tensor_copy(
                                qT[b][h][h * DH:(h + 1) * DH, c0:c0 + 512],
                                ps[h * DH:(h + 1) * DH, :],
                            )
                    else:
                        nc.vector.tensor_copy(dst[:, nkk * 512:(nkk + 1) * 512], ps)
                    yield
                if sec == 2:
                    vT = vt_cur[b]
                    for sc in range(NK):
                        vps = ps_a.tile([128, 128], DTT, tag="psA", name=f"vps{b}_{sc}")
                        nc.tensor.transpose(vps, vT[:, sc * 128:(sc + 1) * 128], ident)
                        vt = vo_pool.tile([128, HPC, 128], DT, tag="vo", name=f"vo{b}_{sc}")
                        nc.vector.tensor_copy(
                            vt[:, :, 0:DH], vps.rearrange("p (h d) -> p h d", h=HPC)
                        )
                        nc.vector.tensor_copy(vt[:, :, DH:DH + 1], ones2)
                        nc.vector.tensor_copy(vt[:, :, DH + 1:], zeros2)
                        vo[b][sc] = vt
                        yield

        vt_cur = {}

        def qkv_start(b, wq_sb, xts):
            if ROW_TILE_QK:
                qT[b] = qk_pool.tile([128, S], DT, tag="qk", name=f"qT{b}")
            else:
                qT[b] = [
                    qk_pool.tile([128, S], DT, tag="qk", name=f"qT{b}_{h}")
                    for h in range(HPC)
                ]
                for h in range(HPC):
                    r0 = DH * (1 - h)
                    for c in range(4):
                        nc.vector.tensor_copy(
                            qT[b][h][r0:r0 + DH, c * 512:(c + 1) * 512], zpad
                        )
            kT[b] = qk_pool.tile([128, S], DT, tag="qk", name=f"kT{b}")
            vo[b] = [None] * NK
            return qkv_chains(b, wq_sb, xts, [(1, range(4)), (2, range(4)), (0, range(4))])

        def attention(b, fillers, rates):
            # fillers: {qh: generator} appended to the live filler at that block
            live = []
            stage[b] = stage_pool.tile([128, S], DT, tag="stg", name=f"stg{b}")

            def filler_step(n):
                for _ in range(n):
                    while live:
                        if next(live[0], StopIteration) is StopIteration:
                            live.pop(0)
                        else:
                            break
                    if not live:
                        return

            for qh in range(NQH):
                q0 = qh * 512
                rate = rates[qh]
                if fillers.get(qh) is not None:
                    live.append(fillers[qh])
                outT = [
                    ps_ot.tile([128, 512], f32, tag="psOT", name=f"oT{b}_{qh}_{h}")
                    for h in range(HPC)
                ]

                def sv(k, st):
                    for h in range(HPC):
                        nc.tensor.matmul(
                            outT[h],
                            lhsT=vo[b][k][:, h, :],
                            rhs=st[:, h * 512:(h + 1) * 512],
                            start=(k == 0),
                            stop=(k == NK - 1),
                        )

                pending = None
                for k in range(NK):
                    lt = ps_lt.tile([128, 1024], f32, tag="psLT", name=f"lt{b}_{qh}_{k}")
                    for h in range(HPC):
                        if ROW_TILE_QK:
                            nc.tensor.matmul(
                                lt[:, h * 512:(h + 1) * 512],
                                lhsT=kT[b][h * DH:(h + 1) * DH, k * 128:(k + 1) * 128],
                                rhs=qT[b][h * DH:(h + 1) * DH, q0:q0 + 512],
                                start=True,
                                stop=True,
                                tile_position=(h * DH, 0),
                            )
                        else:
                            nc.tensor.matmul(
                                lt[:, h * 512:(h + 1) * 512],
                                lhsT=kT[b][:, k * 128:(k + 1) * 128],
                                rhs=qT[b][h][:, q0:q0 + 512],
                                start=True,
                                stop=True,
                            )
                    st = st_pool.tile([128, 1024], DT, tag="st", name=f"st{b}_{qh}_{k}")
                    nc.scalar.activation(st, lt, EXP)
                    if pending is not None:
                        sv(*pending)
                    pending = (k, st)
                    filler_step(rate)
                sv(*pending)
                # normalize both heads into stage (evac on DVE, not ScalarE:
                # ScalarE is the exp-bottleneck engine)
                for h in range(HPC):
                    ot = ot_pool.tile([DH + 1, 512], f32, tag="ot", name=f"ot{b}_{qh}_{h}")
                    nc.vector.tensor_copy(ot, outT[h][0:DH + 1, :])
                    recip = rcp_pool.tile([1, 512], f32, tag="rcp", name=f"rcp{b}_{qh}_{h}")
                    nc.vector.reciprocal(recip, ot[DH:DH + 1, :])
                    bc = bc_pool.tile([DH, 512], f32, tag="bc", name=f"bc{b}_{qh}_{h}")
                    nc.gpsimd.partition_broadcast(bc, recip)
                    nc.vector.scalar_tensor_tensor(
                        out=stage[b][h * DH:(h + 1) * DH, q0:q0 + 512],
                        in0=ot[0:DH, :],
                        scalar=SCALE_INV,
                        in1=bc,
                        op0=mybir.AluOpType.mult,
                        op1=mybir.AluOpType.mult,
                    )
                if qh in (1, 3):
                    half = qh // 2
                    for s in range(NCORES):
                        nc.gpsimd.dma_start(
                            out=a2a_in[b, half][s],
                            in_=stage[b][:, half * 1024 + s * 128:half * 1024 + (s + 1) * 128],
                        )
                    nc.gpsimd.collective_compute(
                        "AllToAll",
                        mybir.AluOpType.bypass,
                        replica_groups=[list(range(NCORES))],
                        ins=[a2a_in[b, half].opt()],
                        outs=[a2a_out[b, half].opt()],
                    )

        def proj_gen(b, half, wo_sb, bias_sb, g_pool, y_pool):
            g_sb = []
            for s in range(NCORES):
                t = g_pool.tile([128, 128], DT, tag="g", name=f"g{b}_{half}_{s}")
                nc.sync.dma_start(out=t, in_=a2a_out[b, half][s])
                g_sb.append(t)

            def gen():
                y_sb = y_pool.tile([128, D], f32, tag="y", name=f"y{b}_{half}")
                for nk in range(2):
                    yps = ps_a.tile([128, 512], f32, tag="psA", name=f"yps{b}_{half}_{nk}")
                    for s in range(NCORES):
                        nc.tensor.matmul(
                            yps,
                            lhsT=g_sb[s],
                            rhs=wo_sb[:, s * D + nk * 512:s * D + (nk + 1) * 512],
                            start=(s == 0),
                            stop=(s == NCORES - 1),
                        )
                        yield
                    nc.vector.tensor_add(
                        y_sb[:, nk * 512:(nk + 1) * 512],
                        yps,
                        bias_sb[:, nk * 512:(nk + 1) * 512],
                    )
                    yield
                nc.sync.dma_start(out=out_ext[b, half], in_=y_sb)
                yield

            return gen()

        with tc.tile_pool(name="xt", bufs=16) as xt_pool, \
                tc.tile_pool(name="wq", bufs=1) as wq_pool:
            wq_sb = wq_pool.tile([128, KC * 3 * 128], DT, tag="wq", name="wq")
            nc.sync.dma_start(out=wq_sb, in_=wq_ext.ap())
            # tiny throwaway AllToAll: the first collective pays ~30us of
            # cold-start; absorb it under the DMA-gated QKV phase
            nc.gpsimd.dma_start(out=cc_warm_in[0:1], in_=wq_sb[0:1, 0:128])
            nc.gpsimd.collective_compute(
                "AllToAll",
                mybir.AluOpType.bypass,
                replica_groups=[list(range(NCORES))],
                ins=[cc_warm_in.opt()],
                outs=[cc_warm_out.opt()],
            )
            xts0 = load_x(0, xt_pool)
            drain(qkv_start(0, wq_sb, xts0))
            xts1 = load_x(1, xt_pool)
            g1 = qkv_start(1, wq_sb, xts1)
            attention(0, {0: g1}, rates=(3, 3, 3, 3))
            drain(g1)
        # xt/wq freed: projection pools fit alongside the attention pools
        wo_pool = ctx.enter_context(tc.tile_pool(name="wo", bufs=1))
        g_pool = ctx.enter_context(tc.tile_pool(name="g", bufs=4 * NCORES))
        y_pool = ctx.enter_context(tc.tile_pool(name="y", bufs=2))
        bias_sb = load_bias(y_pool)
        wo_sb = wo_pool.tile([128, KC * D], DT, tag="wo", name="wo")
        nc.sync.dma_start(out=wo_sb, in_=wo_ext.ap())
        # proj(b0) half0's A2A completed mid-attn(0); half1's completes
        # ~20us into attn(1) - only queue its matmuls from qh2 on so the PE
        # stream never blocks on an in-flight collective
        attention(1, {
            0: proj_gen(0, 0, wo_sb, bias_sb, g_pool, y_pool),
            2: proj_gen(0, 1, wo_sb, bias_sb, g_pool, y_pool),
        }, rates=(1, 1, 1, 1))
        drain(proj_gen(1, 0, wo_sb, bias_sb, g_pool, y_pool))
        drain(proj_gen(1, 1, wo_sb, bias_sb, g_pool, y_pool))

    nc.finalize()
    return nc


def _prep_in_maps(x, w_qkv, w_out, b_out):
    if USE_BF16:
        import ml_dtypes
        dt = ml_dtypes.bfloat16
    else:
        dt = np.float32
    x = np.asarray(x, dtype=np.float32)
    w_qkv = np.asarray(w_qkv, dtype=np.float32)
    w_out = np.asarray(w_out, dtype=np.float32)
    b_out = np.ascontiguousarray(b_out, dtype=np.float32)

    xT = np.ascontiguousarray(
        np.stack([x[0].T, x[1].T]).reshape(2, KC, 128, S).astype(dt)
    )
    # w_out rows grouped per sender s (heads 2s, 2s+1) = natural row order,
    # packed so the whole thing is ONE [128, 8192] transfer
    wo = np.ascontiguousarray(
        w_out.reshape(KC, 128, D).transpose(1, 0, 2).reshape(128, KC * D).astype(dt)
    )
    in_maps = []
    for c in range(NCORES):
        c0 = c * HPC * DH
        shard = np.concatenate(
            [
                w_qkv[:, c0:c0 + 128],
                w_qkv[:, D + c0:D + c0 + 128],
                w_qkv[:, 2 * D + c0:2 * D + c0 + 128],
            ],
            axis=1,
        )  # [1024, 384]
        wq = np.ascontiguousarray(
            shard.reshape(KC, 128, 3 * 128).transpose(1, 0, 2).reshape(128, -1).astype(dt)
        )
        in_maps.append({"xT": xT, "w_qkv": wq, "w_out": wo, "b_out": b_out})
    return in_maps


def _run(x, w_qkv, w_out, b_out, trace=False):
    _ensure_paths()
    from concourse.bass_utils import run_bass_kernel_spmd

    if "nc" not in _CACHE:
        _CACHE["nc"] = _build_nc()
    nc = _CACHE["nc"]
    in_maps = _prep_in_maps(x, w_qkv, w_out, b_out)
    res = run_bass_kernel_spmd(nc, in_maps, list(range(NCORES)), trace=trace)
    out = np.empty((2, S, D), dtype=np.float32)
    for c in range(NCORES):
        o = np.asarray(res.results[c]["out"], dtype=np.float32)
        for b in range(2):
            for half in range(2):
                t0 = half * 1024 + c * 128
                out[b, t0:t0 + 128, :] = o[b, half]
    return out, res


def kernel(x, w_qkv, w_out, b_out):
    out, _ = _run(x, w_qkv, w_out, b_out, trace=False)
    return out
